# revision 2
# baseline (speedup 1.0000x reference)
"""Trainium2 Bass kernel for nn_BaselineLSTM (B=4096, T=512, H=128, S=48).

v2 — restructured from the v1 baseline for lower ACT busy, fewer DMAs and a
shorter per-step critical path.

Strategy (pure data parallel over 8 cores, 512 batch rows per core):
  Host prep:
    - Per timestep-pair packed input block [100, B_local] bf16:
        rows 0-47  x_statics(even t), 48-95 x_statics(odd t),
        row  96    y_flow(even), 97 y_flow(odd), 98 ones, 99 pad.
      ONE DMA per pair (~0.5 DMA/step), issued from the idle Pool engine.
    - Encoder layer 1 is ONE block-diagonal matmul per pair; y_flow rides
      through it via passthrough outputs relu(f) - relu(-f) = f, the ones
      row via relu(1) = 1, so biases ride the gate matmuls with no extra
      DMA or memset.
    - enc_w2 folded into gate weights; autoregressive phase folds the pred
      feedback into W_hh' = w_hh + outer(w_ih[:,0], out_w).
  Device (fully unrolled over 511 steps, two interleaved chains of B/2):
    - PE queue per step: [x-mms A, x-mms B, pred(t-1), h-mms A, h-mms B]
      so the h-independent x-matmuls run during the previous step's tail.
    - Gates PSUM [128, 4*bc] fp32 per chain, quarter order [g, i, f, o],
      pool bufs=3 (6 banks) + encoder (1) + preds (1) = 8 banks.
    - ACT per step: tanh(g) per chain, sigmoid(i,f) per chain, deferred
      sigmoid(o) per chain, ONE merged tanh(c) over both chains.
    - h/c state double-buffered (t%2) so writes never wait on readers.
    - pred_t = out_w @ h_t as one M=1 matmul [1, B_local] into PSUM row
      (t%4)*32, flushed every 4 steps by one DVE copy + one Pool DMA.
"""

import math

import numpy as np
import ml_dtypes

BF16 = ml_dtypes.bfloat16
NCORES = 8
S = 48
H = 128
LA = 3           # lookahead, in timestep-pairs, of the encoder pipeline
SIG_MERGE = False    # True: one sigmoid [i,f,o] per chain; False: [i,f] + [o]
MERGED_TC = False    # tanh(c) over both chains in one instruction
ALT_PARITY = False   # alternate chain emission order by step parity
PRIO_PUSH = 20       # deprioritize enc-scalar/pred-copy by this much
ENC_ON_ACT = True    # encoder bias+relu on ScalarE instead of DVE


# --------------------------------------------------------------------------
# host-side prep
# --------------------------------------------------------------------------

def _prep_weights(enc_w1, enc_b1, enc_w2, enc_b2, w_ih, w_hh, b_ih, b_hh,
                  out_w, out_b):
    """Build folded device weight arrays (shared by all cores)."""
    f32 = np.float32
    enc_w1, enc_b1 = np.asarray(enc_w1, f32), np.asarray(enc_b1, f32)
    enc_w2, enc_b2 = np.asarray(enc_w2, f32), np.asarray(enc_b2, f32)
    w_ih, w_hh = np.asarray(w_ih, f32), np.asarray(w_hh, f32)
    b_ih, b_hh = np.asarray(b_ih, f32), np.asarray(b_hh, f32)
    out_w, out_b = np.asarray(out_w, f32), np.asarray(out_b, f32)

    # torch gate order in w_ih/w_hh rows: i, f, g, o.  PSUM quarter order:
    # [g, i, f, o] (g first so its Tanh can start earliest; i,f,o contiguous
    # for merged sigmoids).
    sel = [2, 0, 1, 3]

    # xin rows: 0-47 relu1(even), 48-95 relu1(odd), 96 relu(f_e),
    # 97 relu(-f_e), 98 relu(f_o), 99 relu(-f_o), 100 ones.
    # Parity is selected by zero-padded weight variants (K=128 both).
    # Phases: 0 teacher-forced, 1 autoregressive (pred fold), 2 t==0 autoreg.
    w50 = np.zeros((3, 2, H, 4 * H), f32)   # [phase, parity, k, q*128+m]
    whh = np.zeros((2, H, 4 * H), f32)      # [phase, k, q*128+m]
    for qi, blk in enumerate(sel):
        r = slice(blk * H, (blk + 1) * H)
        cols = slice(qi * H, (qi + 1) * H)
        wihE = w_ih[r, 1:1 + S]                       # [128, 48]
        w2g = (wihE @ enc_w2).T                       # [48, 128] lhsT rows
        wflow = w_ih[r, 0]                            # [128]
        bias = b_ih[r] + b_hh[r] + wihE @ enc_b2
        for par in (0, 1):
            krow = slice(par * S, (par + 1) * S)
            for ph in (0, 1, 2):
                w50[ph, par, krow, cols] = w2g
            fr = 96 + 2 * par
            w50[0, par, fr, cols] = wflow
            w50[0, par, fr + 1, cols] = -wflow
            w50[0, par, 100, cols] = bias
            w50[1, par, 100, cols] = bias + wflow * out_b[0]
            w50[2, par, 100, cols] = bias
        whh[0, :, cols] = w_hh[r].T
        whh[1, :, cols] = (w_hh[r] + np.outer(wflow, out_w[0])).T

    # encoder stationary [K=128 pairblk rows, M=128 xin rows]
    w1T = np.zeros((H, H), f32)
    w1T[0:S, 0:S] = enc_w1.T
    w1T[S:2 * S, S:2 * S] = enc_w1.T
    w1T[96, 96] = 1.0
    w1T[96, 97] = -1.0
    w1T[97, 98] = 1.0
    w1T[97, 99] = -1.0
    w1T[98, 100] = 1.0
    b1s = np.zeros((H, 1), f32)
    b1s[0:S, 0] = enc_b1
    b1s[S:2 * S, 0] = enc_b1

    return {
        "w50": w50.astype(BF16),
        "whh": whh.astype(BF16),
        "w1T": w1T.astype(BF16),
        "b1s": b1s,
        "outwT": out_w[0][:, None].astype(BF16),      # [128, 1]
    }


def _prep_core_inputs(y_flow, x_statics, b_local, tm1, core):
    """Per-core packed pair blocks."""
    rows = slice(core * b_local, (core + 1) * b_local)
    npairs = math.ceil(tm1 / 2)
    xs = np.asarray(x_statics[rows, :tm1, :], np.float32)       # [b,tm1,48]
    xst = xs.transpose(1, 2, 0)                                 # [tm1,48,b]
    yf = np.asarray(y_flow[rows, :tm1, 0], np.float32).T        # [tm1,b]
    pb = np.zeros((npairs, 128, b_local), np.float32)
    pb[:, 0:S] = xst[0::2]
    n_odd = tm1 // 2
    pb[:n_odd, S:2 * S] = xst[1::2]
    pb[:, 96] = yf[0::2]
    pb[:n_odd, 97] = yf[1::2]
    pb[:, 98] = 1.0
    return {"pairblk": pb.astype(BF16)}


# --------------------------------------------------------------------------
# device program
# --------------------------------------------------------------------------

def build_program(b_local=512, tm1=511, ti=255, reps=1):
    """Build + compile the Bass program.

    ti: number of teacher-forced steps (flow_t is teacher for t < ti).
    reps: repeat the whole computation (timing builds only).
    """
    import concourse.bacc as bacc
    import concourse.mybir as mybir
    import concourse.tile as tile

    dt = mybir.dt
    AF = mybir.ActivationFunctionType
    OP = mybir.AluOpType

    bc = b_local // 2                    # sub-batch (chain) width
    npairs = math.ceil(tm1 / 2)

    nc = bacc.Bacc("TRN2", debug=False, enable_asserts=False,
                   num_devices=NCORES)

    pairblk = nc.dram_tensor("pairblk", [npairs, 128, b_local], dt.bfloat16,
                             kind="ExternalInput").ap()
    w50 = nc.dram_tensor("w50", [3, 2, H, 4 * H], dt.bfloat16,
                         kind="ExternalInput").ap()
    whh = nc.dram_tensor("whh", [2, H, 4 * H], dt.bfloat16,
                         kind="ExternalInput").ap()
    w1T = nc.dram_tensor("w1T", [H, H], dt.bfloat16,
                         kind="ExternalInput").ap()
    b1s = nc.dram_tensor("b1s", [H, 1], dt.float32,
                         kind="ExternalInput").ap()
    outwT = nc.dram_tensor("outwT", [H, 1], dt.bfloat16,
                           kind="ExternalInput").ap()
    tm1_pad = 4 * math.ceil(tm1 / 4)
    preds = nc.dram_tensor("preds", [tm1_pad, b_local], dt.float32,
                           kind="ExternalOutput").ap()

    with tile.TileContext(nc) as tc:
        with tc.tile_pool(name="const", bufs=1) as cp:
            # resident weights
            w50_sb = []
            for p in range(3):
                row = []
                for par in range(2):
                    wt = cp.tile([H, 4 * H], dt.bfloat16,
                                 name=f"w50sb{p}{par}")
                    nc.sync.dma_start(wt[:], w50[p, par])
                    row.append(wt)
                w50_sb.append(row)
            whh_sb = []
            for p in range(2):
                wt = cp.tile([H, 4 * H], dt.bfloat16, name=f"whhsb{p}")
                nc.sync.dma_start(wt[:], whh[p])
                whh_sb.append(wt)
            w1T_sb = cp.tile([H, H], dt.bfloat16, name="w1Tsb")
            nc.sync.dma_start(w1T_sb[:], w1T[:])
            b1s_sb = cp.tile([H, 1], dt.float32)
            nc.sync.dma_start(b1s_sb[:], b1s[:])
            outw_sb = cp.tile([H, 1], dt.bfloat16)
            nc.sync.dma_start(outw_sb[:], outwT[:])

            # double-buffered state: h(t) and c(t) live in tile t%2
            h_st = [cp.tile([H, b_local], dt.bfloat16, name=f"hst{k}")
                    for k in range(2)]
            c_st = [cp.tile([H, b_local], dt.bfloat16, name=f"cst{k}")
                    for k in range(2)]

            for rep in range(reps):
                for tl in h_st + c_st:
                    nc.vector.memset(tl[:], 0.0)

                with tc.tile_pool(name="pax", bufs=3) as pax, \
                     tc.tile_pool(name="pbx", bufs=2 * LA + 2) as pbx, \
                     tc.tile_pool(name="pbs", bufs=4) as pbs, \
                     tc.tile_pool(name="pbt", bufs=8) as pbt, \
                     tc.tile_pool(name="pbth", bufs=2) as pbth, \
                     tc.tile_pool(name="pbo", bufs=2) as pbo, \
                     tc.tile_pool(name="paps1", bufs=1, space="PSUM") as paps1, \
                     tc.tile_pool(name="pbg", bufs=3, space="PSUM") as pbg, \
                     tc.tile_pool(name="pbp", bufs=1, space="PSUM") as pbp:

                    pair_tiles = {}

                    # persistent pred-accumulator PSUM tile (one bank);
                    # memset once so flush copies never read uninit rows
                    predps = pbp.tile([128, b_local], dt.float32,
                                      name="predps")
                    nc.vector.memset(predps[:], 0.0)

                    def emit_pair(p):
                        """Packed DMA + encoder layer1 (+passthrough) for the
                        timestep pair (2p, 2p+1) -> xin tile."""
                        xs2 = pax.tile([128, b_local], dt.bfloat16,
                                       name="xs2")
                        nc.gpsimd.dma_start(xs2[:], pairblk[p])
                        ps1 = paps1.tile([128, b_local], dt.float32,
                                         name="ps1")
                        nc.tensor.matmul(ps1[:], w1T_sb[:],
                                         xs2[:], start=True, stop=True)
                        xin2 = pbx.tile([128, b_local], dt.bfloat16,
                                        name="xin2")
                        with tc.high_priority(offset=-PRIO_PUSH):
                            if ENC_ON_ACT:
                                nc.scalar.activation(xin2[:],
                                                     ps1[:], AF.Relu,
                                                     bias=b1s_sb[:])
                            else:
                                nc.vector.tensor_scalar(
                                    xin2[:], ps1[:],
                                    b1s_sb[:], 0.0, OP.add, OP.max)
                        pair_tiles[p] = xin2

                    def gate_x(t, c, g_ps, q):
                        ph = 0 if t < ti else (2 if t == 0 else 1)
                        par = t % 2
                        xin2 = pair_tiles[t // 2]
                        nc.tensor.matmul(
                            g_ps[:, q * bc:(q + 1) * bc],
                            w50_sb[ph][par][:, q * H:(q + 1) * H],
                            xin2[:, c * bc:(c + 1) * bc],
                            start=True, stop=False)

                    def gate_h(t, c, g_ps, q):
                        phh = 0 if t < ti else 1
                        hprev = h_st[(t + 1) % 2]
                        nc.tensor.matmul(
                            g_ps[:, q * bc:(q + 1) * bc],
                            whh_sb[phh][:, q * H:(q + 1) * H],
                            hprev[:, c * bc:(c + 1) * bc],
                            start=False, stop=True)

                    def emit_gates(t, chains):
                        """All gate matmuls for step t, both chains.

                        PSUM-bank rule: one open accumulation group per 2KB
                        bank.  Quarter layout [g,i,f,o] puts (g,i) in bank 0
                        and (f,o) in bank 1 of each chain's tile, so open
                        (g,f) across both chains first (4 distinct banks, all
                        h-independent), close them with the recurrent mms,
                        then do (i,o)."""
                        g_pss = {}
                        for c in chains:
                            g_pss[c] = pbg.tile([H, 4 * bc], dt.float32,
                                                name="gps")
                        for c in chains:
                            gate_x(t, c, g_pss[c], 0)
                            gate_x(t, c, g_pss[c], 2)
                        for c in chains:
                            gate_h(t, c, g_pss[c], 0)
                            gate_h(t, c, g_pss[c], 2)
                            gate_x(t, c, g_pss[c], 1)
                            gate_h(t, c, g_pss[c], 1)
                            gate_x(t, c, g_pss[c], 3)
                            gate_h(t, c, g_pss[c], 3)
                        return [g_pss[0], g_pss[1]]

                    def emit_pred(t, predps):
                        prow = (t % 4) * 32
                        nc.tensor.matmul(
                            predps[prow:prow + 1, :],
                            outw_sb[:], h_st[t % 2][:], start=True,
                            stop=True, tile_position=(0, prow))

                    def flush_preds(tp, predps):
                        g0 = (tp // 4) * 4
                        nrow = tp - g0 + 1
                        psb = pbo.tile([128, b_local], dt.float32,
                                       name="psb")
                        with tc.high_priority(offset=-PRIO_PUSH):
                            nc.vector.tensor_copy(
                                psb[:(nrow - 1) * 32 + 1, :],
                                predps[:(nrow - 1) * 32 + 1, :])
                        psb4 = psb.rearrange("(a b) f -> a b f", b=32)
                        nc.gpsimd.dma_start(preds[g0:g0 + nrow, :],
                                            psb4[0:nrow, 0, :])

                    # ---- prologue ----
                    for p in range(min(LA, npairs)):
                        emit_pair(p)

                    # ---- main loop ----
                    for t in range(tm1):
                        cprev = c_st[(t + 1) % 2]
                        cnew = c_st[t % 2]
                        hnew = h_st[t % 2]

                        chains = [0, 1]
                        if ALT_PARITY and t % 2 == 1:
                            chains = [1, 0]
                        g_pss = emit_gates(t, chains)
                        # pred for the PREVIOUS step (reads the other h tile,
                        # off the critical path -> after the h matmuls)
                        if t > 0:
                            emit_pred(t - 1, predps)
                        # ACT: tanh(g) + sigmoid per chain; tanh(c) late so
                        # the other chain's ACT work hides the h turnaround
                        sigs = [None, None]
                        tgs = [None, None]
                        for c in chains:
                            g_ps = g_pss[c]
                            tg = pbt.tile([H, bc], dt.bfloat16, name="tg")
                            nc.scalar.activation(tg[:], g_ps[:, 0:bc],
                                                 AF.Tanh)
                            sig = pbs.tile([H, 3 * bc], dt.bfloat16,
                                           name="sig")
                            if SIG_MERGE:
                                nc.scalar.activation(sig[:, 0:3 * bc],
                                                     g_ps[:, bc:4 * bc],
                                                     AF.Sigmoid)
                            else:
                                nc.scalar.activation(sig[:, 0:2 * bc],
                                                     g_ps[:, bc:3 * bc],
                                                     AF.Sigmoid)
                            sigs[c] = sig
                            tgs[c] = tg
                        if not SIG_MERGE:
                            for c in chains:
                                nc.scalar.activation(
                                    sigs[c][:, 2 * bc:3 * bc],
                                    g_pss[c][:, 3 * bc:4 * bc], AF.Sigmoid)
                        # DVE: c update per chain
                        for c in chains:
                            sig, tg = sigs[c], tgs[c]
                            cs = slice(c * bc, (c + 1) * bc)
                            v_t = pbt.tile([H, bc], dt.bfloat16, name="vt")
                            nc.vector.tensor_tensor(v_t[:], sig[:, bc:2 * bc],
                                                    cprev[:, cs], OP.mult)
                            u_t = pbt.tile([H, bc], dt.bfloat16, name="ut")
                            nc.vector.tensor_tensor(u_t[:], tg[:],
                                                    sig[:, 0:bc], OP.mult)
                            nc.vector.tensor_tensor(cnew[:, cs], v_t[:],
                                                    u_t[:], OP.add)
                        # ACT: tanh(c)
                        ths = [None, None]
                        if MERGED_TC:
                            th = pbth.tile([H, b_local], dt.bfloat16,
                                           name="th")
                            nc.scalar.activation(th[:], cnew[:], AF.Tanh)
                            ths = [th[:, 0:bc], th[:, bc:2 * bc]]
                        else:
                            for c in chains:
                                th = pbth.tile([H, bc], dt.bfloat16,
                                               name="th")
                                nc.scalar.activation(
                                    th[:], cnew[:, c * bc:(c + 1) * bc],
                                    AF.Tanh)
                                ths[c] = th[:]
                        # DVE: h update per chain
                        for c in chains:
                            cs = slice(c * bc, (c + 1) * bc)
                            nc.vector.tensor_tensor(hnew[:, cs], ths[c],
                                                    sigs[c][:, 2 * bc:3 * bc],
                                                    OP.mult)
                        # preds flush (after group row 3 written)
                        if t > 0 and (t - 1) % 4 == 3:
                            flush_preds(t - 1, predps)
                        # encoder lookahead, one pair per two steps
                        if t % 2 == 0 and t // 2 + LA < npairs:
                            emit_pair(t // 2 + LA)

                    # ---- epilogue ----
                    emit_pred(tm1 - 1, predps)
                    flush_preds(tm1 - 1, predps)

    nc.compile()
    return nc


# --------------------------------------------------------------------------
# entry point
# --------------------------------------------------------------------------

_PROGRAM_CACHE = {}


def _get_program(b_local, tm1, ti, reps=1):
    key = (b_local, tm1, ti, reps)
    if key not in _PROGRAM_CACHE:
        _PROGRAM_CACHE[key] = build_program(b_local, tm1, ti, reps)
    return _PROGRAM_CACHE[key]


def make_in_maps(y_flow, x_statics, weights, b_local, tm1):
    in_maps = []
    for core in range(NCORES):
        m = dict(weights)
        m.update(_prep_core_inputs(y_flow, x_statics, b_local, tm1, core))
        in_maps.append(m)
    return in_maps


def assemble_output(results, out_b, b_local, tm1):
    B = b_local * NCORES
    out = np.empty((B, tm1, 1), np.float32)
    for core, res in enumerate(results):
        p = res["preds"][:tm1]                               # [tm1, b_local]
        out[core * b_local:(core + 1) * b_local, :, 0] = p.T
    out += np.float32(np.asarray(out_b, np.float32)[0])
    return out


def kernel(y_flow, x_statics, enc_w1, enc_b1, enc_w2, enc_b2,
           w_ih, w_hh, b_ih, b_hh, out_w, out_b, twin_idx):
    from concourse.bass_utils import run_bass_kernel_spmd

    y_flow = np.asarray(y_flow)
    x_statics = np.asarray(x_statics)
    B, T, _ = y_flow.shape
    tm1 = T - 1
    assert B % NCORES == 0
    b_local = B // NCORES
    ti = int(np.clip(int(twin_idx) - 1, 0, tm1))

    nc = _get_program(b_local, tm1, ti)
    weights = _prep_weights(enc_w1, enc_b1, enc_w2, enc_b2, w_ih, w_hh,
                            b_ih, b_hh, out_w, out_b)
    in_maps = make_in_maps(y_flow, x_statics, weights, b_local, tm1)
    res = run_bass_kernel_spmd(nc, in_maps, core_ids=list(range(NCORES)))
    return assemble_output(res.results, out_b, b_local, tm1)
